# revision 4
# baseline (speedup 1.0000x reference)
"""Trainium2 Bass kernel for KMeans assignment (argmin over 8192 centroids).

Problem: x [32768, 1024] f32, centroids [1024, 8192] f32 ->
         argmin_k ||x_n - c_k||^2  as int32 [32768].

Math: argmin_k (||x||^2 - 2 x.c_k + ||c_k||^2) == argmax_k (x.c_k - 0.5*||c_k||^2).

v4 (fixes v3's HAM collapse): fp8-e4m3 DoubleRow matmuls with NB=1 --
all of x^T resident in SBUF (fp8 halves it to 32KB/partition), so there
is no mid-kernel block boundary: v3's two ~4us PE gaps at the b0->b1
transition dropped the PE clock gate to K=4/8 for the remaining 692us
and it never re-warmed. With a single block the PE stream never pauses.

Also: PSUM banks are paired -- each psum tile is [128, 1024] (2 banks),
filled by two 512-col accumulation groups (4 DoubleRow MMs + 1 f32r
bias MM each); DVE then runs one max8 + one max_index over the full
1024 columns, halving per-instruction overhead and the DVE's PSUM-read
count. Host merges per-1024-chunk top-8 candidates, exact-rescoring the
global top-16 in fp64 (verified 0 mismatches in simulation).
"""
import numpy as np

# ---- problem constants (hardcoded per harness contract) ----
N_FULL, D, K = 32768, 1024, 8192
N_CORES = 8
NC = N_FULL // N_CORES          # 4096 rows per core
NT = NC // 128                  # 32 row-tiles
CHUNK = 512                     # one matmul / PSUM bank
PAIR = 2 * CHUNK                # 1024: one psum tile, max8 span
KCM = K // PAIR                 # 8 chunk-pairs
DC = D // 128                   # 8 contraction chunks
DP = DC // 2                    # 4 DoubleRow pairs

_compiled = {}


def _build():
    """Build + compile the per-core Bass program. Returns the Bass object."""
    from contextlib import ExitStack
    import concourse.bacc as bacc
    import concourse.mybir as mybir
    import concourse.tile as tile

    f32 = mybir.dt.float32
    f32r = mybir.dt.float32r
    fp8 = mybir.dt.float8e4
    u32 = mybir.dt.uint32
    DR = mybir.MatmulPerfMode.DoubleRow

    nc = bacc.Bacc("TRN2", target_bir_lowering=False, debug=False)

    xt_d = nc.dram_tensor("xt", [D, NC], fp8, kind="ExternalInput").ap()
    c_d = nc.dram_tensor("cent", [D, K], fp8, kind="ExternalInput").ap()
    bias2_d = nc.dram_tensor("bias2", [1, 2, K], fp8,
                             kind="ExternalInput").ap()
    onesq_d = nc.dram_tensor("onesq", [1, 2, 128], fp8,
                             kind="ExternalInput").ap()
    outv_d = nc.dram_tensor("outv", [KCM, 128, NT * 8], f32,
                            kind="ExternalOutput").ap()
    outi_d = nc.dram_tensor("outi", [KCM, 128, NT * 8], u32,
                            kind="ExternalOutput").ap()

    with tile.TileContext(nc) as tc:
        with ExitStack() as ctx:
            const_pool = ctx.enter_context(tc.tile_pool(name="const", bufs=1))
            xt_pool = ctx.enter_context(tc.tile_pool(name="xt", bufs=1))
            c_pool = ctx.enter_context(tc.tile_pool(name="cent", bufs=2))
            acc_pool = ctx.enter_context(tc.tile_pool(name="acc", bufs=2))
            ps_pool = ctx.enter_context(
                tc.tile_pool(name="psum", bufs=4, space="PSUM"))

            bias2_sb = const_pool.tile([1, 2, K], fp8, name="bias2_sb")
            nc.sync.dma_start(bias2_sb[:], bias2_d[:])
            onesq_sb = const_pool.tile([1, 2, 128], fp8, name="onesq_sb")
            nc.sync.dma_start(onesq_sb[:], onesq_d[:])

            # full x^T resident: [128, DC, NC] fp8 = 32KB/partition
            xt_sb = xt_pool.tile([128, DC, NC], fp8, name="xt_sb")
            for d in range(DC):
                nc.sync.dma_start(xt_sb[:, d, :],
                                  xt_d[d * 128:(d + 1) * 128, :])

            for kg in range(KCM // 2):
                # two kcp chunk-pairs per group: each stationary x^T pair
                # feeds 4 moving streams (one LDWEIGHTS per 4 matmuls)
                c_sb = c_pool.tile([128, DC, 2 * PAIR], fp8, name="c_sb",
                                   tag="c")
                for d in range(DC):
                    nc.sync.dma_start(
                        c_sb[:, d, :],
                        c_d[d * 128:(d + 1) * 128,
                            kg * 2 * PAIR:(kg + 1) * 2 * PAIR])

                mvs = [acc_pool.tile([128, NT * 8], f32, name=f"mv{t}",
                                     tag=f"mv{t}") for t in range(2)]
                mis = [acc_pool.tile([128, NT * 8], u32, name=f"mi{t}",
                                     tag=f"mi{t}") for t in range(2)]

                for nt in range(NT):
                    pss = [ps_pool.tile([128, PAIR], f32, name="ps")
                           for _ in range(2)]
                    for j in range(DP):
                        for t in range(2):
                            for half in range(2):
                                hs = half * CHUNK
                                nc.tensor.matmul(
                                    pss[t][:, hs:hs + CHUNK],
                                    xt_sb[:, 2 * j:2 * j + 2,
                                          nt * 128:(nt + 1) * 128],
                                    c_sb[:, 2 * j:2 * j + 2,
                                         t * PAIR + hs:t * PAIR + hs + CHUNK],
                                    start=(j == 0),
                                    stop=False,
                                    perf_mode=DR)
                    for t in range(2):
                        kcp = kg * 2 + t
                        for half in range(2):
                            hs = half * CHUNK
                            # bias on PE via DoubleRow:
                            # (1/s)*(bias_hi+bias_lo), mean-centered
                            nc.tensor.matmul(
                                pss[t][:, hs:hs + CHUNK],
                                onesq_sb[:],
                                bias2_sb[:, :, kcp * PAIR + hs:
                                         kcp * PAIR + hs + CHUNK],
                                start=False,
                                stop=True,
                                perf_mode=DR)
                    col = nt * 8
                    for t in range(2):
                        nc.vector.max(mvs[t][:, col:col + 8], pss[t][:])
                        nc.vector.max_index(mis[t][:, col:col + 8],
                                            mvs[t][:, col:col + 8], pss[t][:])

                for t in range(2):
                    nc.sync.dma_start(outv_d[kg * 2 + t], mvs[t][:])
                    nc.sync.dma_start(outi_d[kg * 2 + t], mis[t][:])
    nc.compile()
    return nc


def _get_nc(mode: str = "v4"):
    if mode not in _compiled:
        _compiled[mode] = _build()
    return _compiled[mode]


def _make_in_maps(x, centroids):
    import ml_dtypes
    fp8 = ml_dtypes.float8_e4m3fn
    xt = np.ascontiguousarray(x.T)                       # [D, N]
    bias_row = -0.5 * np.einsum("dk,dk->k", centroids, centroids,
                                dtype=np.float64).astype(np.float32)
    # mean-center (uniform shift leaves argmax unchanged), split into
    # two e4m3 terms at a power-of-2 scale
    bc = (bias_row - bias_row.mean()).astype(np.float64)
    s = 4.0
    while s > 1.0 and np.abs(bc * s).max() > 400.0:
        s /= 2.0
    hi = (bc * s).astype(fp8)
    lo = ((bc * s) - hi.astype(np.float64)).astype(fp8)
    bias2 = np.ascontiguousarray(np.stack([hi, lo]).reshape(1, 2, K))
    onesq = np.full((1, 2, 128), 1.0 / s, dtype=fp8)
    cq = centroids.astype(fp8)
    in_maps = []
    for c in range(N_CORES):
        sl = np.ascontiguousarray(xt[:, c * NC:(c + 1) * NC]).astype(fp8)
        in_maps.append({"xt": sl, "cent": cq, "bias2": bias2,
                        "onesq": onesq})
    return in_maps, bias_row


def _merge_host(outv, outi):
    """[KCM, 128, NT*8] device layout -> [NC, KCM, 8] candidates."""
    vals = outv.reshape(KCM, 128, NT, 8).transpose(2, 1, 0, 3).reshape(
        NC, KCM, 8)
    idxs = outi.reshape(KCM, 128, NT, 8).transpose(2, 1, 0, 3).reshape(
        NC, KCM, 8)
    return vals, idxs


def kernel(x: np.ndarray, centroids: np.ndarray) -> np.ndarray:
    from concourse.bass_utils import run_bass_kernel_spmd

    x = np.asarray(x, dtype=np.float32)
    centroids = np.asarray(centroids, dtype=np.float32)
    nc = _get_nc()

    in_maps, bias_row = _make_in_maps(x, centroids)
    res = run_bass_kernel_spmd(nc, in_maps, core_ids=list(range(N_CORES)))

    out = np.empty(N_FULL, dtype=np.int32)
    for c in range(N_CORES):
        vals, idxs = _merge_host(res.results[c]["outv"],
                                 res.results[c]["outi"])
        gi = _refine(x[c * NC:(c + 1) * NC], centroids, bias_row, vals, idxs)
        out[c * NC:(c + 1) * NC] = gi
    return out


def _refine(xs, centroids, bias_row, vals, idxs, top=16):
    """Re-score each row's top candidates exactly to undo fp8 noise."""
    n = xs.shape[0]
    fv = vals.reshape(n, KCM * 8)
    fi = (idxs.astype(np.int64)
          + (np.arange(KCM) * PAIR)[None, :, None]).reshape(n, KCM * 8)
    part = np.argpartition(-fv, top - 1, axis=1)[:, :top]
    cand = np.take_along_axis(fi, part, axis=1)          # [n, top] global idx
    out = np.empty(n, dtype=np.int32)
    bs = 4096
    for s in range(0, n, bs):
        e = min(s + bs, n)
        cb = cand[s:e]                                   # [b, top]
        cc = centroids.T[cb]                             # [b, top, D]
        sc = np.einsum("bd,btd->bt", xs[s:e], cc, dtype=np.float64)
        sc = sc + bias_row[cb]
        # argmax with ties -> smallest global index (first occurrence in k)
        best = sc.max(axis=1, keepdims=True)
        big = np.where(sc >= best, cb, np.iinfo(np.int64).max)
        out[s:e] = big.min(axis=1).astype(np.int32)
    return out
